# revision 1
# baseline (speedup 1.0000x reference)
"""Trainium2 Bass kernel for PointTransformerDecoderInterp.

Math (per batch b, query q):
  logits[q,a] = -|xyz_q[q]-anchors[a]|^2 / VAR   (softmax over a)
  c[q,:]      = softmax(logits) @ anchor_feats
  occ         = MLP(c)  (fc0 -> relu -> fc1, 5 ResnetBlockFC, out head)

Sharding: 65536 total queries -> 8 cores x 8192 (cores 0-3 batch 0,
cores 4-7 batch 1); anchors/feats/params replicated per batch.

Device layout is fully transposed ([feature_partitions, query_free]):
  - scores via one augmented matmul: K=5 rows [ax,ay,az,an2,1] x
    [50qx,50qy,50qz,-25, C-25*qn2] gives logits^T[a,q] pre-scaled by
    1/VAR with a global exp offset C (softmax-invariant, keeps exp in
    fp32 range).
  - exp on ACT; weight-sum via [128,128]-ones matmul (PE broadcasts
    the sum to all partitions for free); reciprocal on DVE;
    normalization fused into the c-extraction (scalar_tensor_tensor).
  - MLP weights are natural lhsT stationaries; `net` accumulates in
    PSUM across fc1/fcc_i/blk1_i matmuls; biases are host-folded into
    cumulative per-extraction bias vectors applied by ACT/DVE.
  - All matmuls run as float32r (fp32 data, 1 cyc/row at N=512);
    f32r-consumed tiles are produced with dtype float32r so walrus
    sees them as rounded.
  - Constants arrive in 4 grouped DMAs (one per partition-height
    group) to keep per-instruction sync-wait counts low.
"""

import numpy as np
from contextlib import ExitStack

from concourse import bass, mybir, tile
from concourse.bass_utils import run_bass_kernel_spmd

F32 = mybir.dt.float32
F32R = mybir.dt.float32r

VAR = 0.2 ** 2
INV = 1.0 / VAR          # 25
C_OFF = 64.0             # global exp offset, cancels in softmax
B, NQ, NA, DI, H, NB = 2, 32768, 1024, 256, 50, 5
NCORES = 8
QC = B * NQ // NCORES    # 8192 queries per core
NT = 512                 # queries per tile
NTILES = QC // NT        # 16

K12 = 12                 # hi/lo-split augmented score rows
# column offsets inside the grouped const tensors
C5_Q, C5_A, C5_W = 0, QC, QC + NA                     # cst5 [12, C5_W]
CK_AF, CK_W0, CK_W1, CK_FCC, CK_ONE, CK_W = 0, 2048, 2560, 2660, 3160, 3288
C50_B0, C50_B1, C50_WO, C50_W = 0, 250, 500, 501
CB_BL, CB_CBN, CB_B0, CB_OB, CB_W = 0, 2, 8, 13, 14

_CACHE = {}


def _r(x):
    return x.bitcast(F32R)


def _tf32_split(x):
    # hi keeps 10 explicit mantissa bits (exactly representable under the
    # PE's f32r rounding); lo carries the remainder.
    u = x.view(np.uint32)
    h = ((u + np.uint32(0x1000)) & np.uint32(0xFFFFE000)).view(np.float32)
    return h, x - h


def _build_nc():
    nc = bass.Bass()

    p5 = nc.declare_dram_parameter("cst5", [K12, C5_W], F32R, isOutput=False)
    pk = nc.declare_dram_parameter("cst128", [128, CK_W], F32R, isOutput=False)
    p50 = nc.declare_dram_parameter("cst50", [50, C50_W], F32R, isOutput=False)
    pb = nc.declare_dram_parameter("cstb", [128, CB_W], F32, isOutput=False)
    occ_d = nc.declare_dram_parameter("occ", [1, QC], F32, isOutput=True)

    AF = mybir.ActivationFunctionType
    OP = mybir.AluOpType

    with tile.TileContext(nc) as tc, ExitStack() as ctx:
        cpool = ctx.enter_context(tc.tile_pool(name="consts", bufs=1))
        s_pool = ctx.enter_context(tc.tile_pool(name="s", bufs=2))
        cn_pool = ctx.enter_context(tc.tile_pool(name="cn", bufs=2))
        lat_pool = ctx.enter_context(tc.tile_pool(name="lat", bufs=2))
        rb_pool = ctx.enter_context(tc.tile_pool(name="rb", bufs=2))
        rn_pool = ctx.enter_context(tc.tile_pool(name="rn", bufs=3))
        rh_pool = ctx.enter_context(tc.tile_pool(name="rh", bufs=2))

        plog = ctx.enter_context(tc.tile_pool(name="plog", bufs=2, space="PSUM"))
        pc = ctx.enter_context(tc.tile_pool(name="pc", bufs=1, space="PSUM"))
        plat = ctx.enter_context(tc.tile_pool(name="plat", bufs=2, space="PSUM"))
        pw = ctx.enter_context(tc.tile_pool(name="pw", bufs=1, space="PSUM"))
        pnet = ctx.enter_context(tc.tile_pool(name="pnet", bufs=1, space="PSUM"))
        ph = ctx.enter_context(tc.tile_pool(name="ph", bufs=1, space="PSUM"))

        c5 = cpool.tile([K12, C5_W], F32R, tag="c5")
        nc.sync.dma_start(out=c5[:, :], in_=p5[:, :])
        ck = cpool.tile([128, CK_W], F32R, tag="ck")
        nc.sync.dma_start(out=ck[:, :], in_=pk[:, :])
        c50 = cpool.tile([50, C50_W], F32R, tag="c50")
        nc.sync.dma_start(out=c50[:, :], in_=p50[:, :])
        cb = cpool.tile([128, CB_W], F32, tag="cb")
        nc.sync.dma_start(out=cb[:, :], in_=pb[:, :])

        q_aug = c5[:, C5_Q:C5_Q + QC]
        a_aug = c5[:, C5_A:C5_A + NA]
        af_t = ck[:, CK_AF:CK_AF + 2048]
        w0 = ck[:, CK_W0:CK_W0 + 512]
        w1 = ck[:, CK_W1:CK_W1 + 100]
        wfcc = ck[:, CK_FCC:CK_FCC + 500]
        ones_m = ck[:, CK_ONE:CK_ONE + 128]
        wblk0 = c50[:, C50_B0:C50_B0 + 250]
        wblk1 = c50[:, C50_B1:C50_B1 + 250]
        wout = c50[:, C50_WO:C50_WO + 1]
        b_lat = cb[:, CB_BL:CB_BL + 2]
        cbn = cb[0:50, CB_CBN:CB_CBN + 6]
        bblk0 = cb[0:50, CB_B0:CB_B0 + 5]
        ob = cb[0:1, CB_OB:CB_OB + 1]

        occ_full = cpool.tile([1, QC], F32, tag="occ_full")

        # Warm-up ops: absorb const-DMA queue waits on ACT/DVE so later
        # consumers (whose instruction structs have only 1 sync-wait slot)
        # get those waits elided by transitivity.
        warm = cpool.tile([1, 2], F32, tag="warm")
        nc.scalar.activation(warm[0:1, 0:1], cb[0:1, 0:1], AF.Copy,
                             bias=0.0, scale=1.0)
        nc.vector.tensor_scalar_add(warm[0:1, 1:2], cb[0:1, 0:1], 0.0)
        pwarm = plog.tile([1, 256], F32, tag="lg")
        nc.tensor.matmul(pwarm[0:1, :], wout, c50[:, 0:256],
                         start=True, stop=True)

        for t in range(NTILES):
            q0 = t * NT
            qs = q_aug[:, q0:q0 + NT]

            # ---- scores: logits^T[a,q] then exp -> s_tile ----
            s_tile = s_pool.tile([128, 8 * NT], F32R)
            for j in range(8):
                lg = plog.tile([128, NT], F32)
                nc.tensor.matmul(lg[:, :], a_aug[:, 128 * j:128 * (j + 1)],
                                 qs, start=True, stop=True)
                nc.scalar.activation(s_tile[:, NT * j:NT * (j + 1)], lg[:, :],
                                     AF.Exp)

            # ---- weight sum (broadcast to all partitions via ones matrix) ----
            ws = pw.tile([128, NT], F32)
            for j in range(8):
                nc.tensor.matmul(ws[:, :], ones_m,
                                 s_tile[:, NT * j:NT * (j + 1)],
                                 start=(j == 0), stop=(j == 7))
            rb = rb_pool.tile([128, NT], F32)
            nc.vector.reciprocal(rb[:, :], ws[:, :])

            c_sb = cn_pool.tile([128, 2 * NT], F32R)
            for m in range(2):
                ct = pc.tile([128, NT], F32)
                for j in range(8):
                    nc.tensor.matmul(
                        ct[:, :],
                        af_t[:, 256 * j + 128 * m:256 * j + 128 * (m + 1)],
                        s_tile[:, NT * j:NT * (j + 1)],
                        start=(j == 0), stop=(j == 7))
                # c_norm = ct * rb  (extract + normalize)
                nc.vector.tensor_tensor(
                    c_sb[:, NT * m:NT * (m + 1)], ct[:, :], rb[:, :], OP.mult)

            # ---- lat = c_norm @ fc0 + b ----
            lat_sb = lat_pool.tile([128, 2 * NT], F32R)
            rlat_sb = lat_pool.tile([128, 2 * NT], F32R)
            for m in range(2):
                lt = plat.tile([128, NT], F32)
                for k in range(2):
                    nc.tensor.matmul(
                        lt[:, :],
                        w0[:, 256 * k + 128 * m:256 * k + 128 * (m + 1)],
                        c_sb[:, NT * k:NT * (k + 1)],
                        start=(k == 0), stop=(k == 1))
                nc.scalar.activation(lat_sb[:, NT * m:NT * (m + 1)], lt[:, :],
                                     AF.Identity, bias=b_lat[:, m:m + 1])
                nc.scalar.activation(rlat_sb[:, NT * m:NT * (m + 1)], lt[:, :],
                                     AF.Relu, bias=b_lat[:, m:m + 1])

            # ---- net accumulation in PSUM ----
            net = pnet.tile([50, NT], F32)
            for k in range(2):
                nc.tensor.matmul(net[:, :], w1[:, 50 * k:50 * (k + 1)],
                                 rlat_sb[:, NT * k:NT * (k + 1)],
                                 start=(k == 0), stop=False)
            for i in range(NB):
                for k in range(2):
                    nc.tensor.matmul(
                        net[:, :],
                        wfcc[:, 100 * i + 50 * k:100 * i + 50 * (k + 1)],
                        lat_sb[:, NT * k:NT * (k + 1)],
                        start=False, stop=False)
                rn = rn_pool.tile([50, NT], F32R)
                nc.vector.tensor_scalar(rn[:, :], net[:, :],
                                        cbn[:, i:i + 1], 0.0, OP.add, OP.max)
                hp = ph.tile([50, NT], F32)
                nc.tensor.matmul(hp[:, :], wblk0[:, 50 * i:50 * (i + 1)],
                                 rn[:, :], start=True, stop=True)
                rh = rh_pool.tile([50, NT], F32R)
                nc.scalar.activation(rh[:, :], hp[:, :], AF.Relu,
                                     bias=bblk0[:, i:i + 1])
                nc.tensor.matmul(net[:, :], wblk1[:, 50 * i:50 * (i + 1)],
                                 rh[:, :], start=False, stop=(i == NB - 1))

            # ---- occ head ----
            rnf = rn_pool.tile([50, NT], F32R)
            nc.vector.tensor_scalar(rnf[:, :], net[:, :],
                                    cbn[:, 5:6], 0.0, OP.add, OP.max)
            op = ph.tile([1, NT], F32, tag="hp")
            nc.tensor.matmul(op[:, :], wout, rnf[:, :],
                             start=True, stop=True)
            nc.vector.tensor_scalar_add(occ_full[0:1, q0:q0 + NT],
                                        op[:, :], ob)

        nc.sync.dma_start(out=occ_d[0:1, :], in_=occ_full[0:1, :])

    _strip_same_engine_waits(nc)
    return nc


def _strip_same_engine_waits(nc):
    # Walrus instruction structs have very few sync-wait slots (1 for most
    # compute ops).  Engines/DMA-queues execute their streams in order, so a
    # wait already implied by the stream predecessor's completion clock or by
    # another wait on the same instruction is redundant and can be removed.
    import bisect
    prod = {}      # sem -> ([cum values], [VC dicts])
    cum = {}       # sem -> cumulative update count
    last_vc = {}   # stream (sem name) -> VC after last instruction

    def lookup(s, v):
        if s not in prod:
            return None
        cums, vcs = prod[s]
        k = bisect.bisect_left(cums, v)
        return vcs[k] if k < len(cums) else None

    for i in nc.all_instructions():
        si = i.sync_info
        if si is None:
            continue
        ups = [u for u in (si.on_update or [])
               if str(u.update_mode) in ("sem-inc", "sem-add-imm")
               and not u.ant_name.startswith("barrier")]
        stream = ups[0].ant_name if ups else None
        vc = dict(last_vc.get(stream, {})) if stream else {}
        waits = list(si.on_wait or [])
        proc_idx = [k for k, w in enumerate(waits)
                    if str(w.wait_mode) == "sem-ge-imm"
                    and not w.ant_name.startswith("barrier")]
        kept = []
        for k in proc_idx:
            w = waits[k]
            if vc.get(w.ant_name, 0) >= w.wait_value:
                continue
            kept.append(k)
        changed = True
        while changed:
            changed = False
            for k in list(kept):
                w = waits[k]
                for k2 in kept:
                    if k2 == k:
                        continue
                    x = waits[k2]
                    pv = lookup(x.ant_name, x.wait_value)
                    if pv and pv.get(w.ant_name, 0) >= w.wait_value:
                        kept.remove(k)
                        changed = True
                        break
                if changed:
                    break
        new_waits = [w for k, w in enumerate(waits)
                     if k not in proc_idx or k in kept]
        if len(new_waits) != len(waits):
            i.sync_info = mybir.SyncInfo(
                on_wait=new_waits, on_update=list(si.on_update or []))
        for k in proc_idx:
            w = waits[k]
            pv = lookup(w.ant_name, w.wait_value)
            if pv:
                for s2, v2 in pv.items():
                    if vc.get(s2, 0) < v2:
                        vc[s2] = v2
            if vc.get(w.ant_name, 0) < w.wait_value:
                vc[w.ant_name] = w.wait_value
        for u in ups:
            c = cum.get(u.ant_name, 0) + u.update_value
            cum[u.ant_name] = c
            vc[u.ant_name] = max(vc.get(u.ant_name, 0), c)
            cums, vcs = prod.setdefault(u.ant_name, ([], []))
            cums.append(c)
            vcs.append(vc)
        if stream:
            last_vc[stream] = vc


def _host_prep(xyz_q, anchors, anchor_feats, fc0_w, fc0_b, fc1_w, fc1_b,
               fcc_w, fcc_b, blk0_w, blk0_b, blk1_w, blk1_b, out_w, out_b):
    f = np.float32
    # cst128: af_t (per batch) + shared weights
    ck_shared = np.zeros((128, CK_W), f)
    ck_shared[:, CK_W0:CK_W0 + 512] = \
        fc0_w.reshape(2, 128, 256).transpose(1, 0, 2).reshape(128, 512)
    ck_shared[:, CK_W1:CK_W1 + 100] = \
        fc1_w.reshape(2, 128, 50).transpose(1, 0, 2).reshape(128, 100)
    ck_shared[:, CK_FCC:CK_FCC + 500] = np.concatenate(
        [fcc_w[i].reshape(2, 128, 50).transpose(1, 0, 2).reshape(128, 100)
         for i in range(NB)], axis=1)
    ck_shared[:, CK_ONE:CK_ONE + 128] = 1.0

    c50 = np.zeros((50, C50_W), f)
    c50[:, C50_B0:C50_B0 + 250] = blk0_w.transpose(1, 0, 2).reshape(50, 250)
    c50[:, C50_B1:C50_B1 + 250] = blk1_w.transpose(1, 0, 2).reshape(50, 250)
    c50[:, C50_WO] = out_w.reshape(50)

    cbm = np.zeros((128, CB_W), f)
    cbm[:, CB_BL:CB_BL + 2] = fc0_b.reshape(2, 128).T
    run = fc1_b.astype(f).copy()
    for i in range(NB):
        run = run + fcc_b[i]
        cbm[0:50, CB_CBN + i] = run
        run = run + blk1_b[i]
    cbm[0:50, CB_CBN + 5] = run
    cbm[0:50, CB_B0:CB_B0 + 5] = blk0_b.T
    cbm[0, CB_OB] = float(out_b.reshape(-1)[0])

    per_batch = []
    for b in range(B):
        an = anchors[b]
        an2 = np.sum(an * an, axis=1)
        ah, al = _tf32_split(np.ascontiguousarray(an.T, f))
        a2h, a2l = _tf32_split(an2.astype(f))
        one = np.ones(NA, f)
        a_aug = np.stack([ah[0], ah[0], al[0], ah[1], ah[1], al[1],
                          ah[2], ah[2], al[2], a2h, a2l, one], 0)
        ckb = ck_shared.copy()
        ckb[:, CK_AF:CK_AF + 2048] = \
            anchor_feats[b].reshape(8, 128, 256).transpose(1, 0, 2).reshape(128, 2048)
        per_batch.append((np.ascontiguousarray(a_aug, f),
                          np.ascontiguousarray(ckb, f)))

    in_maps = []
    for c in range(NCORES):
        b = c // (NCORES // B)
        qs0 = (c % (NCORES // B)) * QC
        q = xyz_q[b, qs0:qs0 + QC]          # [QC, 3]
        qn2 = np.sum(q * q, axis=1)
        Qh, Ql = _tf32_split(np.ascontiguousarray((2.0 * INV) * q.T, f))
        mi = np.full(QC, -INV, f)
        cst5 = np.empty((K12, C5_W), f)
        cst5[:, C5_Q:C5_Q + QC] = np.stack(
            [Qh[0], Ql[0], Qh[0], Qh[1], Ql[1], Qh[1],
             Qh[2], Ql[2], Qh[2], mi, mi,
             (C_OFF - INV * qn2).astype(f)], 0)
        cst5[:, C5_A:C5_A + NA] = per_batch[b][0]
        in_maps.append(dict(cst5=np.ascontiguousarray(cst5, f),
                            cst128=per_batch[b][1],
                            cst50=c50, cstb=cbm))
    return in_maps


def kernel(**inputs):
    if "nc" not in _CACHE:
        _CACHE["nc"] = _build_nc()
    nc = _CACHE["nc"]
    in_maps = _host_prep(**{k: np.asarray(v, np.float32) for k, v in inputs.items()})
    res = run_bass_kernel_spmd(nc, in_maps, list(range(NCORES)))
    out = np.empty((B, NQ, 1), np.float32)
    for c in range(NCORES):
        b = c // (NCORES // B)
        qs0 = (c % (NCORES // B)) * QC
        out[b, qs0:qs0 + QC, 0] = res.results[c]["occ"][0]
    return out



# revision 12
# speedup vs baseline: 108.8344x; 108.8344x over previous
"""Trainium2 Bass kernel for PointTransformerDecoderInterp.

Math (per batch b, query q):
  logits[q,a] = -|xyz_q[q]-anchors[a]|^2 / VAR   (softmax over a)
  c[q,:]      = softmax(logits) @ anchor_feats
  occ         = MLP(c)  (fc0 -> relu -> fc1, 5 ResnetBlockFC, out head)

Sharding: 65536 total queries -> 8 cores x 8192 (cores 0-3 batch 0,
cores 4-7 batch 1); anchors/feats/params replicated per batch.

Device layout is fully transposed ([feature_partitions, query_free]):
  - scores via one augmented matmul: K=12 hi/lo rows give logits^T[a,q]
    pre-scaled by 1/VAR with a global exp offset C (softmax-invariant).
  - fc0 is folded into the anchor features on the host: M = af @ fc0_w,
    so lat_u[f,q] = sum_a M[a,f] exp_s[a,q] comes straight from the
    attention matmul (the separate c tensor and fc0 matmuls vanish).
  - softmax normalization is deferred past the attention matmul:
    lat = lat_u * (1/S) + b0, applied by DVE tensor ops (relu commutes
    with the positive 1/S scale).
  - exp on ACT; weight-sum via ones matmul (PE broadcasts the sum to
    all partitions); reciprocal on DVE.
  - MLP weights are natural lhsT stationaries; `net` accumulates in
    PSUM across fc1/fcc_i/blk1_i matmuls; cumulative biases applied at
    the rn extractions (DVE), rh relu on ACT.
  - Emission is software-pipelined: tile t's scores/exp/weight-sum/lat
    matmuls interleave with tile t-1's MLP so the in-order PE queue
    always has ready work.
  - All matmuls float32r (1 cyc/row at N=512); consts in 4 grouped DMAs.
"""

import numpy as np
from contextlib import ExitStack

from concourse import bass, mybir, tile
from concourse.bass_utils import run_bass_kernel_spmd

F32 = mybir.dt.float32
F32R = mybir.dt.float32r

VAR = 0.2 ** 2
INV = 1.0 / VAR          # 25
C_OFF = 64.0             # global exp offset, cancels in softmax
B, NQ, NA, DI, H, NB = 2, 32768, 1024, 256, 50, 5
NCORES = 8
QC = B * NQ // NCORES    # 8192 queries per core
NT = 512                 # queries per tile
NTILES = QC // NT        # 16

K12 = 12                 # hi/lo-split augmented score rows
# column offsets inside the grouped const tensors
C5_Q, C5_A, C5_W = 0, QC, QC + NA                     # cst5 [12, C5_W]
CK_ONE, CK_MT, CK_W1, CK_FCC, CK_W = 0, 128, 2176, 2276, 2776
CK_SPLIT = CK_MT + 1024   # DMA A covers ones + mt chunks 0-3
C50_B0, C50_B1, C50_WO, C50_W = 0, 250, 500, 501
CB_BL, CB_CBN, CB_B0, CB_OB, CB_W = 0, 2, 8, 13, 14

_CACHE = {}


def _tf32_split(x):
    # hi keeps 10 explicit mantissa bits (exactly representable under the
    # PE's f32r rounding); lo carries the remainder.
    u = x.view(np.uint32)
    h = ((u + np.uint32(0x1000)) & np.uint32(0xFFFFE000)).view(np.float32)
    return h, x - h


def _build_nc():
    nc = bass.Bass()

    p5 = nc.declare_dram_parameter("cst5", [K12, C5_W], F32R, isOutput=False)
    pk = nc.declare_dram_parameter("cst128", [128, CK_W], F32R, isOutput=False)
    p50 = nc.declare_dram_parameter("cst50", [50, C50_W], F32R, isOutput=False)
    pb = nc.declare_dram_parameter("cstb", [128, CB_W], F32, isOutput=False)
    occ_d = nc.declare_dram_parameter("occ", [1, QC], F32, isOutput=True)

    AF = mybir.ActivationFunctionType
    OP = mybir.AluOpType

    with tile.TileContext(nc) as tc, ExitStack() as ctx:
        cpool = ctx.enter_context(tc.tile_pool(name="consts", bufs=1))
        s_pool = ctx.enter_context(tc.tile_pool(name="s", bufs=2))
        rb_pool = ctx.enter_context(tc.tile_pool(name="rb", bufs=2))
        t_pool = ctx.enter_context(tc.tile_pool(name="t", bufs=2))
        lat_pool = ctx.enter_context(tc.tile_pool(name="lat", bufs=3))
        rn_pool = ctx.enter_context(tc.tile_pool(name="rn", bufs=3))
        rh_pool = ctx.enter_context(tc.tile_pool(name="rh", bufs=2))

        plog = ctx.enter_context(tc.tile_pool(name="plog", bufs=2, space="PSUM"))
        pw = ctx.enter_context(tc.tile_pool(name="pw", bufs=1, space="PSUM"))
        plat = ctx.enter_context(tc.tile_pool(name="plat", bufs=2, space="PSUM"))
        pnet = ctx.enter_context(tc.tile_pool(name="pnet", bufs=2, space="PSUM"))
        ph = ctx.enter_context(tc.tile_pool(name="ph", bufs=1, space="PSUM"))

        c5 = cpool.tile([K12, C5_W], F32R, tag="c5")
        nc.sync.dma_start(out=c5[:, :], in_=p5[:, :])
        cb = cpool.tile([128, CB_W], F32, tag="cb")
        nc.sync.dma_start(out=cb[:, :], in_=pb[:, :])
        ck = cpool.tile([128, CK_W], F32R, tag="ck")
        nc.sync.dma_start(out=ck[:, 0:CK_SPLIT], in_=pk[:, 0:CK_SPLIT])
        c50 = cpool.tile([50, C50_W], F32R, tag="c50")
        nc.sync.dma_start(out=c50[:, :], in_=p50[:, :])
        nc.sync.dma_start(out=ck[:, CK_SPLIT:], in_=pk[:, CK_SPLIT:])

        q_aug = c5[:, C5_Q:C5_Q + QC]
        a_aug = c5[:, C5_A:C5_A + NA]
        mt = ck[:, CK_MT:CK_MT + 2048]
        w1 = ck[:, CK_W1:CK_W1 + 100]
        wfcc = ck[:, CK_FCC:CK_FCC + 500]
        ones_m = ck[:, CK_ONE:CK_ONE + 128]
        wblk0 = c50[:, C50_B0:C50_B0 + 250]
        wblk1 = c50[:, C50_B1:C50_B1 + 250]
        wout = c50[:, C50_WO:C50_WO + 1]
        b_lat = cb[:, CB_BL:CB_BL + 2]
        cbn = cb[0:50, CB_CBN:CB_CBN + 6]
        bblk0 = cb[0:50, CB_B0:CB_B0 + 5]
        ob = cb[0:1, CB_OB:CB_OB + 1]

        occ_full = cpool.tile([1, QC], F32, tag="occ_full")

        # Warm-up ops: absorb const-DMA queue waits per engine so later
        # consumers (whose instruction structs have only 1 sync-wait slot)
        # get those waits elided by transitivity.
        warm = cpool.tile([1, 2], F32, tag="warm")
        nc.vector.tensor_scalar_add(warm[0:1, 0:1], cb[0:1, 0:1], 0.0)
        nc.scalar.activation(warm[0:1, 1:2], cb[0:1, 0:1], AF.Copy,
                             bias=0.0, scale=1.0)
        pwarm = plog.tile([1, 256], F32, tag="lg")
        nc.tensor.matmul(pwarm[0:1, :], c5[:, 0:1], c5[:, 0:256],
                         start=True, stop=True)

        # ---- per-tile stages --------------------------------------------
        st = [dict() for _ in range(NTILES)]

        def emit_scores(t):
            # scores matmuls + first 3 exps; s_tile[a, q] = exp(l + C)
            d = st[t]
            d["s"] = s_pool.tile([128, 8 * NT], F32R, name="s", tag="s")
            d["lg"] = []
            qs = q_aug[:, t * NT:t * NT + NT]
            for j in range(8):
                lg = plog.tile([128, NT], F32, name="lg", tag="lg")
                nc.tensor.matmul(lg[:, :], a_aug[:, 128 * j:128 * (j + 1)],
                                 qs, start=True, stop=True)
                d["lg"].append(lg)

        def emit_exp(t, j):
            d = st[t]
            nc.scalar.activation(d["s"][:, NT * j:NT * (j + 1)],
                                 d["lg"][j][:, :], AF.Exp)

        def emit_chunk(t, j):
            # weight-sum + lat accumulation for anchor chunk j
            d = st[t]
            if j == 0:
                d["ws"] = pw.tile([128, NT], F32, name="ws", tag="ws")
                d["lu"] = [plat.tile([128, NT], F32, name=f"lu{m}", tag="lu") for m in range(2)]
            sj = d["s"][:, NT * j:NT * (j + 1)]
            nc.tensor.matmul(d["ws"][:, :], ones_m, sj,
                             start=(j == 0), stop=(j == 7))
            for m in range(2):
                nc.tensor.matmul(
                    d["lu"][m][:, :],
                    mt[:, 256 * j + 128 * m:256 * j + 128 * (m + 1)],
                    sj, start=(j == 0), stop=(j == 7))

        def emit_recip(t):
            d = st[t]
            d["rb"] = rb_pool.tile([128, NT], F32, name="rb", tag="rb")
            nc.vector.reciprocal(d["rb"][:, :], d["ws"][:, :])

        def emit_lat(t):
            # lat = lat_u * rb + b0 ; rlat = relu(lat)   (DVE)
            d = st[t]
            tt = t_pool.tile([128, 2 * NT], F32, tag="tt")
            d["lat"] = lat_pool.tile([128, 2 * NT], F32R, name="lat", tag="lat")
            d["rlat"] = lat_pool.tile([128, 2 * NT], F32R, name="rlat", tag="rlat")
            for m in range(2):
                sl = slice(NT * m, NT * (m + 1))
                nc.vector.tensor_tensor(tt[:, sl], d["lu"][m][:, :],
                                        d["rb"][:, :], OP.mult)
                nc.vector.tensor_scalar(d["lat"][:, sl], tt[:, sl],
                                        b_lat[:, m:m + 1], 0.0,
                                        OP.add, OP.add)
                nc.vector.tensor_scalar(d["rlat"][:, sl], tt[:, sl],
                                        b_lat[:, m:m + 1], 0.0,
                                        OP.add, OP.max)

        def emit_fc1(t):
            d = st[t]
            d["net"] = pnet.tile([50, NT], F32, name="net", tag="net")
            for k in range(2):
                nc.tensor.matmul(d["net"][:, :], w1[:, 50 * k:50 * (k + 1)],
                                 d["rlat"][:, NT * k:NT * (k + 1)],
                                 start=(k == 0), stop=False)

        def emit_fcc(t, i):
            d = st[t]
            for k in range(2):
                nc.tensor.matmul(
                    d["net"][:, :],
                    wfcc[:, 100 * i + 50 * k:100 * i + 50 * (k + 1)],
                    d["lat"][:, NT * k:NT * (k + 1)],
                    start=False, stop=False)

        def emit_rn(t, i):
            d = st[t]
            d["rn"] = rn_pool.tile([50, NT], F32R, name="rn", tag="rn")
            nc.vector.tensor_scalar(d["rn"][:, :], d["net"][:, :],
                                    cbn[:, i:i + 1], 0.0, OP.add, OP.max)

        def emit_blk0(t, i):
            d = st[t]
            d["hp"] = ph.tile([50, NT], F32, name="hp", tag="hp")
            nc.tensor.matmul(d["hp"][:, :], wblk0[:, 50 * i:50 * (i + 1)],
                             d["rn"][:, :], start=True, stop=True)

        def emit_rh(t, i, eng):
            # rh = max(hp + b0_i, 0); hp is PSUM so only ACT/DVE can read it
            d = st[t]
            d["rh"] = rh_pool.tile([50, NT], F32R, name="rh", tag="rh")
            if eng == "act":
                nc.scalar.activation(d["rh"][:, :], d["hp"][:, :], AF.Relu,
                                     bias=bblk0[:, i:i + 1])
            else:
                nc.vector.tensor_scalar(d["rh"][:, :], d["hp"][:, :],
                                        bblk0[:, i:i + 1], 0.0,
                                        OP.add, OP.max)

        def emit_blk1(t, i):
            d = st[t]
            nc.tensor.matmul(d["net"][:, :], wblk1[:, 50 * i:50 * (i + 1)],
                             d["rh"][:, :], start=False, stop=(i == NB - 1))

        def emit_out(t):
            d = st[t]
            d["op"] = ph.tile([1, NT], F32, name="op", tag="hp")
            nc.tensor.matmul(d["op"][:, :], wout, d["rn"][:, :],
                             start=True, stop=True)

        def emit_occ(t):
            d = st[t]
            nc.vector.tensor_scalar_add(
                occ_full[0:1, t * NT:t * NT + NT], d["op"][:, :], ob)

        # ---- software-pipelined emission (3 stages) ---------------------
        # iteration it: front of tile f, fc1+blocks 0-2 of m1=f-1,
        # blocks 3-4 + head of m2=f-2.  The two MLP chains are interleaved
        # so each chain's cross-engine waits overlap the other chain's work.
        # ACT runs only exps; rh on Pool; rn/recip/lat/head on DVE.
        for it in range(NTILES + 2):
            f = it if it < NTILES else None
            m1 = it - 1 if 0 <= it - 1 < NTILES else None
            m2 = it - 2 if 0 <= it - 2 < NTILES else None

            if f is not None:
                emit_scores(f)
                emit_exp(f, 0)
                emit_exp(f, 1)
            if m2 is not None:
                emit_fcc(m2, 3)
                emit_rn(m2, 3)
            if m1 is not None:
                emit_fc1(m1)
            if f is not None:
                emit_chunk(f, 0)
                emit_exp(f, 2)
            if m2 is not None:
                emit_blk0(m2, 3)
                emit_rh(m2, 3, "act")
            if m1 is not None:
                emit_fcc(m1, 0)
                emit_rn(m1, 0)
            if f is not None:
                emit_chunk(f, 1)
                emit_exp(f, 3)
            if m2 is not None:
                emit_blk1(m2, 3)
                emit_fcc(m2, 4)
                emit_rn(m2, 4)
            if m1 is not None:
                emit_blk0(m1, 0)
                emit_rh(m1, 0, "dve")
            if f is not None:
                emit_chunk(f, 2)
                emit_exp(f, 4)
            if m2 is not None:
                emit_blk0(m2, 4)
                emit_rh(m2, 4, "act")
            if m1 is not None:
                emit_blk1(m1, 0)
                emit_fcc(m1, 1)
                emit_rn(m1, 1)
            if f is not None:
                emit_chunk(f, 3)
                emit_exp(f, 5)
            if it == 0:
                # PE warm touching c50 so the blk0/blk1/out matmuls don't
                # carry the c50 DMA wait (matmuls have one sync-wait slot).
                pwarm2 = ph.tile([1, 256], F32, name="pwarm2", tag="hp")
                nc.tensor.matmul(pwarm2[0:1, :], wout, c50[:, 0:256],
                                 start=True, stop=True)
            if m2 is not None:
                emit_blk1(m2, 4)
                emit_rn(m2, 5)
            if m1 is not None:
                emit_blk0(m1, 1)
                emit_rh(m1, 1, "dve")
            if f is not None:
                emit_chunk(f, 4)
                emit_exp(f, 6)
            if m2 is not None:
                emit_out(m2)
            if m1 is not None:
                emit_blk1(m1, 1)
                emit_fcc(m1, 2)
                emit_rn(m1, 2)
            if f is not None:
                emit_chunk(f, 5)
                emit_exp(f, 7)
            if m2 is not None:
                emit_occ(m2)
            if m1 is not None:
                emit_blk0(m1, 2)
                emit_rh(m1, 2, "dve")
            if f is not None:
                emit_chunk(f, 6)
            if m1 is not None:
                emit_blk1(m1, 2)
            if f is not None:
                emit_chunk(f, 7)
                emit_recip(f)
                emit_lat(f)

        nc.sync.dma_start(out=occ_d[0:1, :], in_=occ_full[0:1, :])

    _strip_same_engine_waits(nc)
    return nc


def _strip_same_engine_waits(nc):
    # Walrus instruction structs have very few sync-wait slots (1 for most
    # compute ops).  Engines/DMA-queues execute their streams in order, so a
    # wait already implied by the stream predecessor's completion clock or by
    # another wait on the same instruction is redundant and can be removed.
    import bisect
    prod = {}      # sem -> ([cum values], [VC dicts])
    cum = {}       # sem -> cumulative update count
    last_vc = {}   # stream (sem name) -> VC after last instruction

    def lookup(s, v):
        if s not in prod:
            return None
        cums, vcs = prod[s]
        k = bisect.bisect_left(cums, v)
        return vcs[k] if k < len(cums) else None

    for i in nc.all_instructions():
        si = i.sync_info
        if si is None:
            continue
        ups = [u for u in (si.on_update or [])
               if str(u.update_mode) in ("sem-inc", "sem-add-imm")
               and not u.ant_name.startswith("barrier")]
        stream = ups[0].ant_name if ups else None
        vc = dict(last_vc.get(stream, {})) if stream else {}
        waits = list(si.on_wait or [])
        proc_idx = [k for k, w in enumerate(waits)
                    if str(w.wait_mode) == "sem-ge-imm"
                    and not w.ant_name.startswith("barrier")]
        kept = []
        for k in proc_idx:
            w = waits[k]
            if vc.get(w.ant_name, 0) >= w.wait_value:
                continue
            kept.append(k)
        changed = True
        while changed:
            changed = False
            for k in list(kept):
                w = waits[k]
                for k2 in kept:
                    if k2 == k:
                        continue
                    x = waits[k2]
                    pv = lookup(x.ant_name, x.wait_value)
                    if pv and pv.get(w.ant_name, 0) >= w.wait_value:
                        kept.remove(k)
                        changed = True
                        break
                if changed:
                    break
        new_waits = [w for k, w in enumerate(waits)
                     if k not in proc_idx or k in kept]
        if len(new_waits) != len(waits):
            i.sync_info = mybir.SyncInfo(
                on_wait=new_waits, on_update=list(si.on_update or []))
        for k in proc_idx:
            w = waits[k]
            pv = lookup(w.ant_name, w.wait_value)
            if pv:
                for s2, v2 in pv.items():
                    if vc.get(s2, 0) < v2:
                        vc[s2] = v2
            if vc.get(w.ant_name, 0) < w.wait_value:
                vc[w.ant_name] = w.wait_value
        for u in ups:
            c = cum.get(u.ant_name, 0) + u.update_value
            cum[u.ant_name] = c
            vc[u.ant_name] = max(vc.get(u.ant_name, 0), c)
            cums, vcs = prod.setdefault(u.ant_name, ([], []))
            cums.append(c)
            vcs.append(vc)
        if stream:
            last_vc[stream] = vc


def _host_prep(xyz_q, anchors, anchor_feats, fc0_w, fc0_b, fc1_w, fc1_b,
               fcc_w, fcc_b, blk0_w, blk0_b, blk1_w, blk1_b, out_w, out_b):
    f = np.float32
    # cst128: mt = (anchor_feats @ fc0_w) per batch + shared weights
    ck_shared = np.zeros((128, CK_W), f)
    ck_shared[:, CK_ONE:CK_ONE + 128] = 1.0
    ck_shared[:, CK_W1:CK_W1 + 100] = \
        fc1_w.reshape(2, 128, 50).transpose(1, 0, 2).reshape(128, 100)
    ck_shared[:, CK_FCC:CK_FCC + 500] = np.concatenate(
        [fcc_w[i].reshape(2, 128, 50).transpose(1, 0, 2).reshape(128, 100)
         for i in range(NB)], axis=1)

    c50 = np.zeros((50, C50_W), f)
    c50[:, C50_B0:C50_B0 + 250] = blk0_w.transpose(1, 0, 2).reshape(50, 250)
    c50[:, C50_B1:C50_B1 + 250] = blk1_w.transpose(1, 0, 2).reshape(50, 250)
    c50[:, C50_WO] = out_w.reshape(50)

    cbm = np.zeros((128, CB_W), f)
    cbm[:, CB_BL:CB_BL + 2] = fc0_b.reshape(2, 128).T
    run = fc1_b.astype(f).copy()
    for i in range(NB):
        run = run + fcc_b[i]
        cbm[0:50, CB_CBN + i] = run
        run = run + blk1_b[i]
    cbm[0:50, CB_CBN + 5] = run
    cbm[0:50, CB_B0:CB_B0 + 5] = blk0_b.T
    cbm[0, CB_OB] = float(out_b.reshape(-1)[0])

    per_batch = []
    for b in range(B):
        an = anchors[b]
        an2 = np.sum(an * an, axis=1)
        ah, al = _tf32_split(np.ascontiguousarray(an.T, f))
        a2h, a2l = _tf32_split(an2.astype(f))
        one = np.ones(NA, f)
        a_aug = np.stack([ah[0], ah[0], al[0], ah[1], ah[1], al[1],
                          ah[2], ah[2], al[2], a2h, a2l, one], 0)
        ckb = ck_shared.copy()
        m_mat = (anchor_feats[b].astype(np.float64)
                 @ fc0_w.astype(np.float64)).astype(f)      # [1024, 256]
        ckb[:, CK_MT:CK_MT + 2048] = \
            m_mat.reshape(8, 128, 256).transpose(1, 0, 2).reshape(128, 2048)
        per_batch.append((np.ascontiguousarray(a_aug, f),
                          np.ascontiguousarray(ckb, f)))

    in_maps = []
    for c in range(NCORES):
        b = c // (NCORES // B)
        qs0 = (c % (NCORES // B)) * QC
        q = xyz_q[b, qs0:qs0 + QC]          # [QC, 3]
        qn2 = np.sum(q * q, axis=1)
        Qh, Ql = _tf32_split(np.ascontiguousarray((2.0 * INV) * q.T, f))
        mi = np.full(QC, -INV, f)
        cst5 = np.empty((K12, C5_W), f)
        cst5[:, C5_Q:C5_Q + QC] = np.stack(
            [Qh[0], Ql[0], Qh[0], Qh[1], Ql[1], Qh[1],
             Qh[2], Ql[2], Qh[2], mi, mi,
             (C_OFF - INV * qn2).astype(f)], 0)
        cst5[:, C5_A:C5_A + NA] = per_batch[b][0]
        in_maps.append(dict(cst5=np.ascontiguousarray(cst5, f),
                            cst128=per_batch[b][1],
                            cst50=c50, cstb=cbm))
    return in_maps


def kernel(**inputs):
    if "nc" not in _CACHE:
        _CACHE["nc"] = _build_nc()
    nc = _CACHE["nc"]
    in_maps = _host_prep(**{k: np.asarray(v, np.float32) for k, v in inputs.items()})
    res = run_bass_kernel_spmd(nc, in_maps, list(range(NCORES)))
    out = np.empty((B, NQ, 1), np.float32)
    for c in range(NCORES):
        b = c // (NCORES // B)
        qs0 = (c % (NCORES // B)) * QC
        out[b, qs0:qs0 + QC, 0] = res.results[c]["occ"][0]
    return out
